# revision 25
# baseline (speedup 1.0000x reference)
"""Trainium2 Bass kernel for nn_CausalSelfAttention_77695958385275.

Self-contained: hardcodes shapes/sharding from the problem spec.

Architecture (8 NeuronCores, tensor-parallel over heads, SPMD-homogeneous):
  core c owns: dense head c, sparse head 8+c, full KV head c//2 (for the
  dense head), strided-only KV head 4+c//2 (for the sparse head).
  Every core runs the identical program; only input data differs.

Per-core pipeline (all matmuls bf16, f32 PSUM):
  QKV(b0) -> strided kv -> attn(b0) -> AllToAll(b0)  [hidden behind...]
  QKV(b1) -> attn(b1) -> AllToAll(b1)                [hidden behind...]
  proj(b0) -> proj(b1)
Attention is software-pipelined (S-tile lookahead) so ACT exp latency
stays off the PE critical path; causal masks are multiplicative 0/1
masks applied on DVE after exp.  The projection keeps the gathered
activations stationary (lhsT) and streams w_proj columns, so every proj
matmul runs at the full 512-row PSUM-bank width with token-major output.

DMAs execute synchronously on their trigger engine (DMA_DIRECT2D), so
queue choice is scheduling: SP carries x chunks / bulk late-needed
weights / collective-gated loads; the scalar (ACT) queue carries only
the small early weights so chunk evictions are never blocked.  All
host-side tensors are pre-laid-out partition-major so every input DMA
moves contiguous per-partition lines.
"""

import math
import ml_dtypes
import numpy as np

import bass_rust
import concourse.bass as bass
import concourse.tile as tile
from concourse import mybir
from concourse.bass_utils import run_bass_kernel_spmd
from concourse.tile import TileContext

# ---------------- problem constants ----------------
B, T, DIM = 2, 2048, 2048
H, KV, HD = 16, 8, 128
NUM_FULL = 8
STRIDE = 45
NS = (T + STRIDE - 1) // STRIDE  # 46 strided keys per batch
SCALE = 1.0 / np.sqrt(np.float32(HD)).astype(np.float32)
N_CORES = 8
BT = B * T  # 4096 tokens total
HALF = HD // 2

F32 = mybir.dt.float32
BF16 = mybir.dt.bfloat16

QCH = 512            # attention q-chunk width
NTCH = T // QCH      # 4 q-chunks per batch
KTILE = 128          # key tile
NKT = QCH // KTILE   # 4 key tiles per q-chunk column
XCH = 512            # qkv token chunk
NCH = BT // XCH      # 8 token chunks
CT = DIM // 128      # 16 contraction tiles
TSL = T // N_CORES   # 256 tokens per rank per batch

ScopedClock = bass_rust.ScopedClock


class SplitDrainTileContext(TileContext):
    """This walrus build allows a single sync-wait slot per CTRL/drain;
    split the tail drain's waits across a chain of single-wait drains."""

    def _drain_and_barrier(self, tick_clock, wait_clock):
        nc = self.nc
        drain_inst = nc.sync.drain()
        wait_clock.add_sem_waits(
            drain_inst.ins, ScopedClock({None: tick_clock.global_clock})
        )
        si = drain_inst.ins.sync_info
        ow = list(si.on_wait or []) if si is not None else []
        if len(ow) > 1:
            si.on_wait = [ow[0]]
            drain_inst.ins.sync_info = si
            for w in ow[1:]:
                d2 = nc.sync.drain()
                s2 = d2.ins.sync_info
                if s2 is None:
                    s2 = bass_rust.SyncInfo(on_wait=[w], on_update=[])
                else:
                    s2.on_wait = [w]
                d2.ins.sync_info = s2
        nc.all_engine_barrier()
        assert self.sems is not None
        popped = nc._tile_sem_poison_stack.pop()
        assert popped is self._sem_poison
        nc.clear_and_free_semaphores(list(self.sems.allocated().values()))
        nc.all_engine_barrier()


def split_multi_waits(nc, max_waits=1):
    """Walrus here rejects >1 sync wait on several instruction formats; move
    extra waits onto preceding same-engine NoOps."""
    for f in nc.m.functions:
        for b in f.blocks:
            new = []
            changed = False
            for inst in b.instructions:
                si = inst.sync_info
                ow = list(si.on_wait) if (si is not None and si.on_wait) else []
                if len(ow) > max_waits:
                    changed = True
                    for w in ow[:-max_waits]:
                        nop = mybir.InstNoOp(
                            name=nc.get_next_instruction_name(), ins=[], outs=[]
                        )
                        nop.engine = inst.engine
                        nop.sync_info = bass_rust.SyncInfo(on_wait=[w], on_update=[])
                        new.append(nop)
                    si.on_wait = ow[-max_waits:]
                    inst.sync_info = si
                new.append(inst)
            if changed:
                b.instructions = new


# ---------------- host-side constant tables ----------------

def _const_tables():
    BF = ml_dtypes.bfloat16
    pos = np.arange(T, dtype=np.float32)
    freqs = (np.arange(HALF, dtype=np.float32) / np.float32(HALF))
    ang = pos[:, None] * freqs[None, :]                  # [T, 64]
    cosv = np.cos(ang.astype(np.float64)).astype(np.float32).T   # [64, T]
    sinv = np.sin(ang.astype(np.float64)).astype(np.float32).T
    cc = np.concatenate([cosv, cosv], axis=0)            # [128, T]
    ssn = np.concatenate([-sinv, sinv], axis=0)          # signed sin for rope
    sp = np.arange(0, T, STRIDE)
    ccS = np.concatenate([cc[:, sp], cc[:, sp]], axis=1)      # [128, 92]
    ssnS = np.concatenate([ssn[:, sp], ssn[:, sp]], axis=1)
    # rotate-half as PE matmul lhsT: rsw[0:64]=dst[64:128], rsw[64:128]=dst[0:64]
    mrotT = np.zeros((HD, HD), np.float32)
    for i in range(HALF):
        mrotT[i + HALF, i] = 1.0
        mrotT[i, i + HALF] = 1.0
    ident = np.eye(128, dtype=np.float32)
    ones = np.ones((128, 128), np.float32)
    # multiplicative causal masks (applied to P after exp)
    tri01 = (np.arange(128)[None, :] >= np.arange(128)[:, None]).astype(
        np.float32)                                      # [key row, q col]
    q = np.arange(T)
    smask01 = (q[None, :] >= (STRIDE * np.arange(NS))[:, None]).astype(
        np.float32)                                      # [46, T]
    c = lambda a: np.ascontiguousarray(a.astype(BF))
    return (c(cc), c(ssn), c(ccS), c(ssnS), c(mrotT), c(ident), c(ones),
            c(tri01), c(smask01))


def _pmajor(w):
    """[CT*128, n] -> [128, CT, n] so the device DMA reads contiguous
    per-partition lines."""
    n = w.shape[1]
    return np.ascontiguousarray(w.reshape(CT, 128, n).transpose(1, 0, 2))


# ---------------- device program ----------------

def build_program():
    nc = bass.Bass(num_devices=N_CORES)

    # all inputs pre-laid-out partition-major on the host
    xP = nc.dram_tensor("xP", [128, NCH, CT, XCH], BF16, kind="ExternalInput")
    xsP = nc.dram_tensor("xsP", [128, CT, B * NS], BF16, kind="ExternalInput")
    wqP = nc.dram_tensor("wqP", [128, CT, 2 * HD], BF16, kind="ExternalInput")
    wkP = nc.dram_tensor("wkP", [128, CT, HD], BF16, kind="ExternalInput")
    wvP = nc.dram_tensor("wvP", [128, CT, HD], BF16, kind="ExternalInput")
    wksP = nc.dram_tensor("wksP", [128, CT, HD], BF16, kind="ExternalInput")
    wvsP = nc.dram_tensor("wvsP", [128, CT, HD], BF16, kind="ExternalInput")
    wpP = nc.dram_tensor("wpP", [128, CT, DIM], BF16, kind="ExternalInput")
    # token-major output: rows b*TSL + local token, cols = model dim
    out_tok = nc.dram_tensor("out_tok", [B * TSL, DIM], F32,
                             kind="ExternalOutput")

    # AllToAll per batch: in rows = 8 blocks of [dense128|sparse128] per
    # destination rank; out rows = same blocks from each source rank
    a2ain = [nc.dram_tensor(f"a2ain{b}", [N_CORES * 2 * HD, TSL], BF16,
                            kind="Internal") for b in range(B)]
    a2aout = [nc.dram_tensor(f"a2aout{b}", [N_CORES * 2 * HD, TSL], BF16,
                             kind="Internal") for b in range(B)]

    wu_in = nc.dram_tensor("wu_in", [64, 64], BF16, kind="Internal")
    wu_out = nc.dram_tensor("wu_out", [64, 64], BF16, kind="Internal")

    cc_h, ssn_h, ccS_h, ssnS_h, mrot_h, ident_h, ones_h, tri_h, smask_h = \
        _const_tables()
    cc_d = nc.inline_tensor(cc_h, "ccT")
    ssn_d = nc.inline_tensor(ssn_h, "ssnT")
    ccS_d = nc.inline_tensor(ccS_h, "ccS")
    ssnS_d = nc.inline_tensor(ssnS_h, "ssnS")
    mrot_d = nc.inline_tensor(mrot_h, "mrot")
    ident_d = nc.inline_tensor(ident_h, "ident")
    ones_d = nc.inline_tensor(ones_h, "onesm")
    tri_d = nc.inline_tensor(tri_h, "trim")
    smask_d = nc.inline_tensor(smask_h, "smask")

    AF = mybir.ActivationFunctionType
    OP = mybir.AluOpType

    with SplitDrainTileContext(nc) as tc:
        with tc.tile_pool(name="pp", bufs=1) as PP, \
             tc.tile_pool(name="wk", bufs=1) as WORK, \
             tc.tile_pool(name="ps", bufs=1, space="PSUM") as PS:
            # persistent SBUF state (all bf16)
            qdT = PP.tile([128, BT], BF16, tag="qdT")    # dense-head q^T
            qsT = PP.tile([128, BT], BF16, tag="qsT")    # sparse-head q^T
            kT = PP.tile([128, BT], BF16, tag="kT")      # full k^T of kv_a
            vtok = PP.tile([128, BT], BF16, tag="vtok")  # v token-major
            ksT = PP.tile([128, B * NS], BF16, tag="ksT")
            vs = PP.tile([NS, B * HD], BF16, tag="vs")
            cc = PP.tile([128, T], BF16, tag="cc")
            ssn = PP.tile([128, T], BF16, tag="ssn")
            mrot = PP.tile([128, 128], BF16, tag="mrot")
            ident = PP.tile([128, 128], BF16, tag="ident")
            ones = PP.tile([128, 128], BF16, tag="ones")
            tri01 = PP.tile([128, 128], BF16, tag="tri01")
            smask = PP.tile([NS, T], BF16, tag="smask")
            ccS = PP.tile([128, B * NS], BF16, tag="ccS")
            ssnS = PP.tile([128, B * NS], BF16, tag="ssnS")
            wq_sb = PP.tile([128, CT, 2 * HD], BF16, tag="wq")
            wk_sb = PP.tile([128, CT, HD], BF16, tag="wk")
            wv_sb = PP.tile([128, CT, HD], BF16, tag="wv")
            wp_sb = PP.tile([128, CT, DIM], BF16, tag="wp")
            xs_sb = PP.tile([128, CT, B * NS], BF16, tag="xs")
            wks_sb = PP.tile([128, CT, HD], BF16, tag="wks")
            wvs_sb = PP.tile([128, CT, HD], BF16, tag="wvs")

            # scalar queue: only the small weights/consts needed first
            nc.scalar.dma_start(wq_sb[:], wqP[:])
            nc.scalar.dma_start(wk_sb[:], wkP[:])
            nc.scalar.dma_start(wv_sb[:], wvP[:])
            nc.scalar.dma_start(mrot[:], mrot_d[:])
            nc.scalar.dma_start(ident[:], ident_d[:])
            nc.scalar.dma_start(ones[:], ones_d[:])
            nc.scalar.dma_start(tri01[:], tri_d[:])
            nc.scalar.dma_start(cc[:], cc_d[:])
            nc.scalar.dma_start(ssn[:], ssn_d[:])

            # no warmup collective: its rank-skew spin throttles the DMA
            # rings to ~20GB/s for its whole window; A2A(b0) triggers at
            # ~165us with >100us of compute cover, absorbing skew for free

            def rope(dst, sl, cct, sst, psl, w):
                """dst[:,sl] = dst[:,sl]*cc[:,psl] + rot(dst[:,sl])*ssn[:,psl]
                rotate-half via a PE matmul (mrot), mults/add on Pool+DVE."""
                rsw = PS.tile([128, XCH], F32, tag="yacc", bufs=2, name="rsw")
                nc.tensor.matmul(rsw[:, 0:w], mrot[:], dst[:, sl],
                                 start=True, stop=True)
                t1 = WORK.tile([128, XCH], F32, tag="t1", bufs=1)
                t2 = WORK.tile([128, XCH], F32, tag="t2", bufs=2)
                nc.gpsimd.tensor_mul(t1[:, 0:w], dst[:, sl], cct[:, psl])
                nc.vector.scalar_tensor_tensor(
                    t2[:, 0:w], rsw[:, 0:w], 1.0, sst[:, psl],
                    op0=OP.mult, op1=OP.mult)
                nc.vector.tensor_add(dst[:, sl], t1[:, 0:w], t2[:, 0:w])

            def qkv_chunk(b, tch):
                chix = b * NTCH + tch
                c0 = chix * XCH
                sl = slice(c0, c0 + XCH)
                pos = slice(tch * XCH, (tch + 1) * XCH)
                x_sb = WORK.tile([128, CT, XCH], BF16, tag="x", bufs=2)
                # halves: the ci-0 matmul can start after the first piece.
                # Alternate hardware queues: collectives throttle the DMA
                # rings, so one queue alone starves the x stream
                dq = nc.scalar if tch in (2,) or (b == 1 and tch in (0, 2)) \
                    else nc.sync
                dq.dma_start(x_sb[:, 0:CT // 2, :],
                             xP[:, chix, 0:CT // 2, :])
                dq.dma_start(x_sb[:, CT // 2:CT, :],
                             xP[:, chix, CT // 2:CT, :])
                for wt, msl, dst in ((wq_sb, slice(0, 128), qdT),
                                     (wq_sb, slice(128, 256), qsT),
                                     (wk_sb, slice(0, HD), kT)):
                    ps = PS.tile([128, XCH], F32, tag="S", bufs=4)
                    for ci in range(CT):
                        nc.tensor.matmul(ps[:], wt[:, ci, msl], x_sb[:, ci, :],
                                         start=(ci == 0), stop=(ci == CT - 1))
                    nc.scalar.copy(dst[:, sl], ps[:])
                    rope(dst, sl, cc, ssn, pos, XCH)
                # v^T then transpose to token-major
                ps = PS.tile([128, XCH], F32, tag="S", bufs=4)
                for ci in range(CT):
                    nc.tensor.matmul(ps[:], wv_sb[:, ci, :], x_sb[:, ci, :],
                                     start=(ci == 0), stop=(ci == CT - 1))
                vt = WORK.tile([128, XCH], BF16, tag="vt", bufs=2)
                nc.scalar.copy(vt[:], ps[:])
                for sub in range(XCH // 128):
                    pt = PS.tile([128, 128], BF16, tag="acc", bufs=2)
                    nc.tensor.transpose(
                        pt[:], vt[:, sub * 128:(sub + 1) * 128], ident[:])
                    j = (c0 // 128) + sub
                    nc.vector.tensor_copy(vtok[:, j * 128:(j + 1) * 128],
                                          pt[:])

            def strided_phase():
                ps = PS.tile([128, B * NS], F32, tag="S", bufs=4)
                for ci in range(CT):
                    nc.tensor.matmul(ps[:], wks_sb[:, ci, :], xs_sb[:, ci, :],
                                     start=(ci == 0), stop=(ci == CT - 1))
                nc.scalar.copy(ksT[:], ps[:])
                for b in range(B):
                    psv = PS.tile([NS, HD], F32, tag="acc", bufs=2)
                    for ci in range(CT):
                        nc.tensor.matmul(
                            psv[:], xs_sb[:, ci, b * NS:(b + 1) * NS],
                            wvs_sb[:, ci, :],
                            start=(ci == 0), stop=(ci == CT - 1))
                    nc.vector.tensor_copy(vs[:, b * HD:(b + 1) * HD], psv[:])
                rope(ksT, slice(0, B * NS), ccS, ssnS, slice(0, B * NS),
                     B * NS)

            def attn_batch(b):
                st = {}

                def emit_S(J, i):
                    cl = max(0, KTILE * i - QCH * J)
                    S = PS.tile([128, QCH], F32, tag="S", bufs=4)
                    nc.tensor.matmul(
                        S[:, cl:QCH],
                        kT[:, b * T + i * KTILE: b * T + (i + 1) * KTILE],
                        qdT[:, b * T + J * QCH + cl: b * T + (J + 1) * QCH],
                        start=True, stop=True)
                    return (J, i, cl, S)

                def finalize(J, sums, yacc, row_off):
                    rs = WORK.tile([128, QCH], F32, tag="rs", bufs=2)
                    nc.vector.reciprocal(rs[:], sums[:])
                    ye = WORK.tile([128, QCH], BF16, tag="ye", bufs=2)
                    nc.vector.scalar_tensor_tensor(
                        ye[:], yacc[:], 1.0, rs[:], op0=OP.mult, op1=OP.mult)
                    for u in range(2):
                        r0 = (2 * J + u) * 2 * HD + row_off
                        nc.gpsimd.dma_start(
                            a2ain[b][r0:r0 + 128, :],
                            ye[:, u * TSL:(u + 1) * TSL])

                def consume(J, i, cl, S, first, last):
                    P = WORK.tile([128, QCH], BF16, tag="P", bufs=4)
                    nc.scalar.activation(P[:, cl:QCH], S[:, cl:QCH], AF.Exp)
                    if KTILE * i >= QCH * J:  # diagonal tile: 0/1 tri mask
                        nc.vector.tensor_mul(P[:, cl:cl + KTILE],
                                             P[:, cl:cl + KTILE], tri01[:])
                    if first:
                        st[J] = (PS.tile([128, QCH], F32, tag="acc", bufs=2,
                                         name="sums"),
                                 PS.tile([128, QCH], F32, tag="yacc", bufs=2,
                                         name="yaccd"))
                    sums, yacc = st[J]
                    nc.tensor.matmul(sums[:, cl:QCH], ones[:], P[:, cl:QCH],
                                     start=first, stop=last,
                                     skip_group_check=True)
                    jj = (b * T) // KTILE + i
                    nc.tensor.matmul(
                        yacc[:, cl:QCH],
                        vtok[:, jj * KTILE:(jj + 1) * KTILE], P[:, cl:QCH],
                        start=first, stop=last, skip_group_check=True)
                    if last:
                        finalize(J, sums, yacc, 0)

                # ---- strided sparse head first (its A2A rows are ready
                # early; the dense J=3 eviction then gates the trigger) ----
                def emit_Ss(J):
                    S = PS.tile([NS, QCH], F32, tag="S", bufs=4)
                    nc.tensor.matmul(
                        S[:], ksT[:, b * NS:(b + 1) * NS],
                        qsT[:, b * T + J * QCH: b * T + (J + 1) * QCH],
                        start=True, stop=True)
                    return (J, S)

                def consume_s(J, S):
                    P = WORK.tile([NS, QCH], BF16, tag="Ps", bufs=2)
                    nc.scalar.activation(P[:], S[:], AF.Exp)
                    nc.vector.tensor_mul(
                        P[:], P[:], smask[:, J * QCH:(J + 1) * QCH])
                    sums = PS.tile([128, QCH], F32, tag="acc", bufs=2)
                    nc.tensor.matmul(sums[:], ones[0:NS, :], P[:],
                                     start=True, stop=True)
                    yacc = PS.tile([128, QCH], F32, tag="yacc", bufs=2)
                    nc.tensor.matmul(yacc[:], vs[:, b * HD:(b + 1) * HD],
                                     P[:], start=True, stop=True)
                    finalize(J, sums, yacc, 128)

                spend = []
                for J in range(NTCH):
                    spend.append(emit_Ss(J))
                    if len(spend) > 2:
                        consume_s(*spend.pop(0))
                while spend:
                    consume_s(*spend.pop(0))

                # ---- dense causal head ----
                # diag tiles first within each J: their exp->mask->sums
                # chain hides behind the full tiles' PE work; the leading
                # tile (i=4J) has cl=0 so start=True covers the full width
                sched = []
                for J in range(NTCH):
                    order = list(range(NKT * J, (J + 1) * NKT)) + \
                        list(range(0, NKT * J))
                    for pos, i in enumerate(order):
                        sched.append((J, i, pos == 0, pos == len(order) - 1))
                pend = []
                for (J, i, first, last) in sched:
                    e = emit_S(J, i)
                    pend.append((e[0], e[1], e[2], e[3], first, last))
                    if len(pend) > 3:
                        consume(*pend.pop(0))
                while pend:
                    consume(*pend.pop(0))

                nc.gpsimd.collective_compute(
                    "AllToAll", OP.bypass,
                    ins=[a2ain[b][:]], outs=[a2aout[b][:]],
                    replica_groups=[list(range(N_CORES))],
                )

            def proj_batch(b):
                ya = WORK.tile([128, CT, TSL], BF16, tag="ya", bufs=2)
                # SP queue: collective-gated load must not block the ACT
                # stream (the scheduler hoists it to the collective's
                # simulated completion, which is optimistic vs hardware)
                nc.sync.dma_start(
                    ya[:], a2aout[b][:].rearrange("(a p) n -> p a n", p=128))
                for oc in range(DIM // 512):
                    for tt in range(TSL // 128):
                        ps = PS.tile([128, 512], F32, tag="S", bufs=4)
                        for ci in range(CT):
                            nc.tensor.matmul(
                                ps[:], ya[:, ci, tt * 128:(tt + 1) * 128],
                                wp_sb[:, ci, oc * 512:(oc + 1) * 512],
                                start=(ci == 0), stop=(ci == CT - 1))
                        oe = WORK.tile([128, 512], F32, tag="oe", bufs=1)
                        nc.scalar.copy(oe[:], ps[:])
                        nc.scalar.dma_start(
                            out_tok[b * TSL + tt * 128: b * TSL + (tt + 1) * 128,
                                    oc * 512:(oc + 1) * 512], oe[:])

            # ---------------- schedule ----------------
            # SP queue priority order: x0 first (first-matmul critical path),
            # then the remaining b0 chunks and strided inputs
            qkv_chunk(0, 0)
            qkv_chunk(0, 1)
            nc.sync.dma_start(xs_sb[:], xsP[:])
            nc.sync.dma_start(wks_sb[:], wksP[:])
            nc.sync.dma_start(wvs_sb[:], wvsP[:])
            nc.sync.dma_start(ccS[:], ccS_d[:])
            nc.sync.dma_start(ssnS[:], ssnS_d[:])
            nc.sync.dma_start(smask[:], smask_d[:])
            qkv_chunk(0, 2)
            # strided kv early: its rope chain hides under chunk 3
            strided_phase()
            qkv_chunk(0, 3)
            nc.sync.dma_start(wp_sb[:], wpP[:])
            attn_batch(0)
            for tch in range(NTCH):
                qkv_chunk(1, tch)
            attn_batch(1)
            proj_batch(0)
            proj_batch(1)

    split_multi_waits(nc)
    return nc


_PROG_CACHE = {}


def _get_program():
    if "nc" not in _PROG_CACHE:
        _PROG_CACHE["nc"] = build_program()
    return _PROG_CACHE["nc"]


def _host_prep(x, w_attn, w_proj, q_gain, attn_temp):
    x = np.asarray(x, np.float32)
    w_attn = np.asarray(w_attn, np.float32)
    w_proj = np.asarray(w_proj, np.float32)
    q_gain = np.asarray(q_gain, np.float32)
    attn_temp = np.asarray(attn_temp, np.float32)

    BF = ml_dtypes.bfloat16
    xT = x.reshape(BT, DIM).T.astype(BF)                             # [DIM, BT]
    # [128, NCH, CT, XCH]: per-partition-contiguous chunk lines
    xP = np.ascontiguousarray(
        xT.reshape(CT, 128, NCH, XCH).transpose(1, 2, 0, 3))
    xs = x[:, ::STRIDE, :]                                           # [B, 46, DIM]
    xsP = _pmajor(xs.reshape(B * NS, DIM).T.astype(BF))

    g = (q_gain * attn_temp * SCALE).astype(np.float32)              # [H]
    wq = w_attn[:H * HD].reshape(H, HD, DIM)
    wq = wq * g[:, None, None]
    wk = w_attn[H * HD:(H + KV) * HD].reshape(KV, HD, DIM)
    wv = w_attn[(H + KV) * HD:].reshape(KV, HD, DIM)

    # w_proj^T with input dims permuted to A2A row order:
    # rank r contributes [dense head r | sparse head 8+r]
    perm = np.concatenate(
        [np.concatenate([np.arange(r * HD, (r + 1) * HD),
                         np.arange((8 + r) * HD, (9 + r) * HD)])
         for r in range(N_CORES)])
    wpP = _pmajor(w_proj.T[perm, :].astype(BF))

    in_maps = []
    for c in range(N_CORES):
        kva, kvb = c // 2, 4 + c // 2
        in_maps.append({
            "xP": xP,
            "xsP": xsP,
            "wqP": _pmajor(
                np.concatenate([wq[c], wq[8 + c]], axis=0).T.astype(BF)),
            "wkP": _pmajor(wk[kva].T.astype(BF)),
            "wvP": _pmajor(wv[kva].T.astype(BF)),
            "wksP": _pmajor(wk[kvb].T.astype(BF)),
            "wvsP": _pmajor(wv[kvb].T.astype(BF)),
            "wpP": wpP,
        })
    return in_maps


def run(x, w_attn, w_proj, q_gain, attn_temp, trace=False):
    nc = _get_program()
    in_maps = _host_prep(x, w_attn, w_proj, q_gain, attn_temp)
    res = run_bass_kernel_spmd(nc, in_maps, core_ids=list(range(N_CORES)),
                               trace=trace)
    out = np.empty((B, T, DIM), np.float32)
    for c in range(N_CORES):
        sh = res.results[c]["out_tok"]                               # [512, DIM]
        for b in range(B):
            out[b, c * TSL:(c + 1) * TSL, :] = sh[b * TSL:(b + 1) * TSL, :]
    return out, res


def kernel(x, w_attn, w_proj, q_gain, attn_temp):
    out, _ = run(x, w_attn, w_proj, q_gain, attn_temp, trace=False)
    return out


# revision 26
# speedup vs baseline: 1.1033x; 1.1033x over previous
"""Trainium2 Bass kernel for nn_CausalSelfAttention_77695958385275.

Self-contained: hardcodes shapes/sharding from the problem spec.

Architecture (8 NeuronCores, tensor-parallel over heads, SPMD-homogeneous):
  core c owns: dense head c, sparse head 8+c, full KV head c//2 (for the
  dense head), strided-only KV head 4+c//2 (for the sparse head).
  Every core runs the identical program; only input data differs.

Per-core pipeline (all matmuls bf16, f32 PSUM):
  QKV(b0) -> strided kv -> attn(b0) -> AllToAll(b0)  [hidden behind...]
  QKV(b1) -> attn(b1) -> AllToAll(b1)                [hidden behind...]
  proj(b0) -> proj(b1)
Attention is software-pipelined (S-tile lookahead) so ACT exp latency
stays off the PE critical path; causal masks are multiplicative 0/1
masks applied on DVE after exp.  The projection keeps the gathered
activations stationary (lhsT) and streams w_proj columns, so every proj
matmul runs at the full 512-row PSUM-bank width with token-major output.

DMAs execute synchronously on their trigger engine (DMA_DIRECT2D), so
queue choice is scheduling: SP carries x chunks / bulk late-needed
weights / collective-gated loads; the scalar (ACT) queue carries only
the small early weights so chunk evictions are never blocked.  All
host-side tensors are pre-laid-out partition-major so every input DMA
moves contiguous per-partition lines.
"""

import math
import ml_dtypes
import numpy as np

import bass_rust
import concourse.bass as bass
import concourse.tile as tile
from concourse import mybir
from concourse.bass_utils import run_bass_kernel_spmd
from concourse.tile import TileContext

# ---------------- problem constants ----------------
B, T, DIM = 2, 2048, 2048
H, KV, HD = 16, 8, 128
NUM_FULL = 8
STRIDE = 45
NS = (T + STRIDE - 1) // STRIDE  # 46 strided keys per batch
SCALE = 1.0 / np.sqrt(np.float32(HD)).astype(np.float32)
N_CORES = 8
BT = B * T  # 4096 tokens total
HALF = HD // 2

F32 = mybir.dt.float32
BF16 = mybir.dt.bfloat16

QCH = 512            # attention q-chunk width
NTCH = T // QCH      # 4 q-chunks per batch
KTILE = 128          # key tile
NKT = QCH // KTILE   # 4 key tiles per q-chunk column
XCH = 512            # qkv token chunk
NCH = BT // XCH      # 8 token chunks
CT = DIM // 128      # 16 contraction tiles
TSL = T // N_CORES   # 256 tokens per rank per batch

ScopedClock = bass_rust.ScopedClock


class SplitDrainTileContext(TileContext):
    """This walrus build allows a single sync-wait slot per CTRL/drain;
    split the tail drain's waits across a chain of single-wait drains."""

    def _drain_and_barrier(self, tick_clock, wait_clock):
        nc = self.nc
        drain_inst = nc.sync.drain()
        wait_clock.add_sem_waits(
            drain_inst.ins, ScopedClock({None: tick_clock.global_clock})
        )
        si = drain_inst.ins.sync_info
        ow = list(si.on_wait or []) if si is not None else []
        if len(ow) > 1:
            si.on_wait = [ow[0]]
            drain_inst.ins.sync_info = si
            for w in ow[1:]:
                d2 = nc.sync.drain()
                s2 = d2.ins.sync_info
                if s2 is None:
                    s2 = bass_rust.SyncInfo(on_wait=[w], on_update=[])
                else:
                    s2.on_wait = [w]
                d2.ins.sync_info = s2
        nc.all_engine_barrier()
        assert self.sems is not None
        popped = nc._tile_sem_poison_stack.pop()
        assert popped is self._sem_poison
        nc.clear_and_free_semaphores(list(self.sems.allocated().values()))
        nc.all_engine_barrier()


def split_multi_waits(nc, max_waits=1):
    """Walrus here rejects >1 sync wait on several instruction formats; move
    extra waits onto preceding same-engine NoOps."""
    for f in nc.m.functions:
        for b in f.blocks:
            new = []
            changed = False
            for inst in b.instructions:
                si = inst.sync_info
                ow = list(si.on_wait) if (si is not None and si.on_wait) else []
                if len(ow) > max_waits:
                    changed = True
                    for w in ow[:-max_waits]:
                        nop = mybir.InstNoOp(
                            name=nc.get_next_instruction_name(), ins=[], outs=[]
                        )
                        nop.engine = inst.engine
                        nop.sync_info = bass_rust.SyncInfo(on_wait=[w], on_update=[])
                        new.append(nop)
                    si.on_wait = ow[-max_waits:]
                    inst.sync_info = si
                new.append(inst)
            if changed:
                b.instructions = new


# ---------------- host-side constant tables ----------------

def _const_tables():
    BF = ml_dtypes.bfloat16
    pos = np.arange(T, dtype=np.float32)
    freqs = (np.arange(HALF, dtype=np.float32) / np.float32(HALF))
    ang = pos[:, None] * freqs[None, :]                  # [T, 64]
    cosv = np.cos(ang.astype(np.float64)).astype(np.float32).T   # [64, T]
    sinv = np.sin(ang.astype(np.float64)).astype(np.float32).T
    cc = np.concatenate([cosv, cosv], axis=0)            # [128, T]
    ssn = np.concatenate([-sinv, sinv], axis=0)          # signed sin for rope
    sp = np.arange(0, T, STRIDE)
    ccS = np.concatenate([cc[:, sp], cc[:, sp]], axis=1)      # [128, 92]
    ssnS = np.concatenate([ssn[:, sp], ssn[:, sp]], axis=1)
    # rotate-half as PE matmul lhsT: rsw[0:64]=dst[64:128], rsw[64:128]=dst[0:64]
    mrotT = np.zeros((HD, HD), np.float32)
    for i in range(HALF):
        mrotT[i + HALF, i] = 1.0
        mrotT[i, i + HALF] = 1.0
    ident = np.eye(128, dtype=np.float32)
    ones = np.ones((128, 128), np.float32)
    # dense causal mask, additive (applied to S in PSUM pre-exp);
    # sparse mask multiplicative (applied to P after exp)
    tri01 = np.where(np.arange(128)[None, :] >= np.arange(128)[:, None],
                     0.0, -1e9).astype(np.float32)       # [key row, q col]
    q = np.arange(T)
    smask01 = (q[None, :] >= (STRIDE * np.arange(NS))[:, None]).astype(
        np.float32)                                      # [46, T]
    c = lambda a: np.ascontiguousarray(a.astype(BF))
    return (c(cc), c(ssn), c(ccS), c(ssnS), c(mrotT), c(ident), c(ones),
            c(tri01), c(smask01))


def _pmajor(w):
    """[CT*128, n] -> [128, CT, n] so the device DMA reads contiguous
    per-partition lines."""
    n = w.shape[1]
    return np.ascontiguousarray(w.reshape(CT, 128, n).transpose(1, 0, 2))


# ---------------- device program ----------------

def build_program():
    nc = bass.Bass(num_devices=N_CORES)

    # all inputs pre-laid-out partition-major on the host
    xP = nc.dram_tensor("xP", [128, NCH, CT, XCH], BF16, kind="ExternalInput")
    xsP = nc.dram_tensor("xsP", [128, CT, B * NS], BF16, kind="ExternalInput")
    wqP = nc.dram_tensor("wqP", [128, CT, 2 * HD], BF16, kind="ExternalInput")
    wkP = nc.dram_tensor("wkP", [128, CT, HD], BF16, kind="ExternalInput")
    wvP = nc.dram_tensor("wvP", [128, CT, HD], BF16, kind="ExternalInput")
    wksP = nc.dram_tensor("wksP", [128, CT, HD], BF16, kind="ExternalInput")
    wvsP = nc.dram_tensor("wvsP", [128, CT, HD], BF16, kind="ExternalInput")
    wpP = nc.dram_tensor("wpP", [128, CT, DIM], BF16, kind="ExternalInput")
    # token-major output: rows b*TSL + local token, cols = model dim
    out_tok = nc.dram_tensor("out_tok", [B * TSL, DIM], F32,
                             kind="ExternalOutput")

    # AllToAll per batch: in rows = 8 blocks of [dense128|sparse128] per
    # destination rank; out rows = same blocks from each source rank
    a2ain = [nc.dram_tensor(f"a2ain{b}", [N_CORES * 2 * HD, TSL], BF16,
                            kind="Internal") for b in range(B)]
    a2aout = [nc.dram_tensor(f"a2aout{b}", [N_CORES * 2 * HD, TSL], BF16,
                             kind="Internal") for b in range(B)]

    wu_in = nc.dram_tensor("wu_in", [64, 64], BF16, kind="Internal")
    wu_out = nc.dram_tensor("wu_out", [64, 64], BF16, kind="Internal")

    cc_h, ssn_h, ccS_h, ssnS_h, mrot_h, ident_h, ones_h, tri_h, smask_h = \
        _const_tables()
    cc_d = nc.inline_tensor(cc_h, "ccT")
    ssn_d = nc.inline_tensor(ssn_h, "ssnT")
    ccS_d = nc.inline_tensor(ccS_h, "ccS")
    ssnS_d = nc.inline_tensor(ssnS_h, "ssnS")
    mrot_d = nc.inline_tensor(mrot_h, "mrot")
    ident_d = nc.inline_tensor(ident_h, "ident")
    ones_d = nc.inline_tensor(ones_h, "onesm")
    tri_d = nc.inline_tensor(tri_h, "trim")
    smask_d = nc.inline_tensor(smask_h, "smask")

    AF = mybir.ActivationFunctionType
    OP = mybir.AluOpType

    with SplitDrainTileContext(nc) as tc:
        with tc.tile_pool(name="pp", bufs=1) as PP, \
             tc.tile_pool(name="wk", bufs=1) as WORK, \
             tc.tile_pool(name="ps", bufs=1, space="PSUM") as PS:
            # persistent SBUF state (all bf16)
            qdT = PP.tile([128, BT], BF16, tag="qdT")    # dense-head q^T
            qsT = PP.tile([128, BT], BF16, tag="qsT")    # sparse-head q^T
            kT = PP.tile([128, BT], BF16, tag="kT")      # full k^T of kv_a
            vtok = PP.tile([128, BT], BF16, tag="vtok")  # v token-major
            ksT = PP.tile([128, B * NS], BF16, tag="ksT")
            vs = PP.tile([NS, B * HD], BF16, tag="vs")
            cc = PP.tile([128, T], BF16, tag="cc")
            ssn = PP.tile([128, T], BF16, tag="ssn")
            mrot = PP.tile([128, 128], BF16, tag="mrot")
            ident = PP.tile([128, 128], BF16, tag="ident")
            ones = PP.tile([128, 128], BF16, tag="ones")
            tri01 = PP.tile([128, 128], BF16, tag="tri01")
            smask = PP.tile([NS, T], BF16, tag="smask")
            ccS = PP.tile([128, B * NS], BF16, tag="ccS")
            ssnS = PP.tile([128, B * NS], BF16, tag="ssnS")
            wq_sb = PP.tile([128, CT, 2 * HD], BF16, tag="wq")
            wk_sb = PP.tile([128, CT, HD], BF16, tag="wk")
            wv_sb = PP.tile([128, CT, HD], BF16, tag="wv")
            wp_sb = PP.tile([128, CT, DIM], BF16, tag="wp")
            xs_sb = PP.tile([128, CT, B * NS], BF16, tag="xs")
            wks_sb = PP.tile([128, CT, HD], BF16, tag="wks")
            wvs_sb = PP.tile([128, CT, HD], BF16, tag="wvs")

            # scalar queue: only the small weights/consts needed first
            nc.scalar.dma_start(wq_sb[:], wqP[:])
            nc.scalar.dma_start(wk_sb[:], wkP[:])
            nc.scalar.dma_start(wv_sb[:], wvP[:])
            nc.scalar.dma_start(mrot[:], mrot_d[:])
            nc.scalar.dma_start(ident[:], ident_d[:])
            nc.scalar.dma_start(ones[:], ones_d[:])
            nc.scalar.dma_start(tri01[:], tri_d[:])
            nc.scalar.dma_start(cc[:], cc_d[:])
            nc.scalar.dma_start(ssn[:], ssn_d[:])

            # no warmup collective: its rank-skew spin throttles the DMA
            # rings to ~20GB/s for its whole window; A2A(b0) triggers at
            # ~165us with >100us of compute cover, absorbing skew for free

            def rope(dst, sl, cct, sst, psl, w):
                """dst[:,sl] = dst[:,sl]*cc[:,psl] + rot(dst[:,sl])*ssn[:,psl]
                rotate-half via a PE matmul (mrot), mults/add on Pool+DVE."""
                rsw = PS.tile([128, XCH], F32, tag="yacc", bufs=2, name="rsw")
                nc.tensor.matmul(rsw[:, 0:w], mrot[:], dst[:, sl],
                                 start=True, stop=True)
                t1 = WORK.tile([128, XCH], F32, tag="t1", bufs=1)
                t2 = WORK.tile([128, XCH], F32, tag="t2", bufs=2)
                nc.gpsimd.tensor_mul(t1[:, 0:w], dst[:, sl], cct[:, psl])
                nc.vector.scalar_tensor_tensor(
                    t2[:, 0:w], rsw[:, 0:w], 1.0, sst[:, psl],
                    op0=OP.mult, op1=OP.mult)
                nc.vector.tensor_add(dst[:, sl], t1[:, 0:w], t2[:, 0:w])

            def qkv_chunk(b, tch):
                chix = b * NTCH + tch
                c0 = chix * XCH
                sl = slice(c0, c0 + XCH)
                pos = slice(tch * XCH, (tch + 1) * XCH)
                x_sb = WORK.tile([128, CT, XCH], BF16, tag="x", bufs=2)
                # halves: the ci-0 matmul can start after the first piece
                nc.sync.dma_start(x_sb[:, 0:CT // 2, :],
                                  xP[:, chix, 0:CT // 2, :])
                nc.sync.dma_start(x_sb[:, CT // 2:CT, :],
                                  xP[:, chix, CT // 2:CT, :])
                for wt, msl, dst in ((wq_sb, slice(0, 128), qdT),
                                     (wq_sb, slice(128, 256), qsT),
                                     (wk_sb, slice(0, HD), kT)):
                    ps = PS.tile([128, XCH], F32, tag="S", bufs=4)
                    for ci in range(CT):
                        nc.tensor.matmul(ps[:], wt[:, ci, msl], x_sb[:, ci, :],
                                         start=(ci == 0), stop=(ci == CT - 1))
                    nc.scalar.copy(dst[:, sl], ps[:])
                    rope(dst, sl, cc, ssn, pos, XCH)
                # v^T then transpose to token-major
                ps = PS.tile([128, XCH], F32, tag="S", bufs=4)
                for ci in range(CT):
                    nc.tensor.matmul(ps[:], wv_sb[:, ci, :], x_sb[:, ci, :],
                                     start=(ci == 0), stop=(ci == CT - 1))
                vt = WORK.tile([128, XCH], BF16, tag="vt", bufs=2)
                nc.scalar.copy(vt[:], ps[:])
                for sub in range(XCH // 128):
                    pt = PS.tile([128, 128], BF16, tag="acc", bufs=2)
                    nc.tensor.transpose(
                        pt[:], vt[:, sub * 128:(sub + 1) * 128], ident[:])
                    j = (c0 // 128) + sub
                    nc.vector.tensor_copy(vtok[:, j * 128:(j + 1) * 128],
                                          pt[:])

            def strided_phase():
                ps = PS.tile([128, B * NS], F32, tag="S", bufs=4)
                for ci in range(CT):
                    nc.tensor.matmul(ps[:], wks_sb[:, ci, :], xs_sb[:, ci, :],
                                     start=(ci == 0), stop=(ci == CT - 1))
                nc.scalar.copy(ksT[:], ps[:])
                for b in range(B):
                    psv = PS.tile([NS, HD], F32, tag="acc", bufs=2)
                    for ci in range(CT):
                        nc.tensor.matmul(
                            psv[:], xs_sb[:, ci, b * NS:(b + 1) * NS],
                            wvs_sb[:, ci, :],
                            start=(ci == 0), stop=(ci == CT - 1))
                    nc.vector.tensor_copy(vs[:, b * HD:(b + 1) * HD], psv[:])
                rope(ksT, slice(0, B * NS), ccS, ssnS, slice(0, B * NS),
                     B * NS)

            def attn_batch(b):
                st = {}

                def emit_S(J, i):
                    cl = max(0, KTILE * i - QCH * J)
                    S = PS.tile([128, QCH], F32, tag="S", bufs=4)
                    diag = KTILE * i >= QCH * J
                    nc.tensor.matmul(
                        S[:, cl:QCH],
                        kT[:, b * T + i * KTILE: b * T + (i + 1) * KTILE],
                        qdT[:, b * T + J * QCH + cl: b * T + (J + 1) * QCH],
                        start=True, stop=not diag)
                    if diag:  # additive -1e9 causal mask, PE-only chain
                        nc.tensor.matmul(S[:, cl:cl + KTILE], ident[:],
                                         tri01[:], start=False, stop=True,
                                         skip_group_check=True)
                    return (J, i, cl, S)

                def finalize(J, sums, yacc, row_off):
                    rs = WORK.tile([128, QCH], F32, tag="rs", bufs=2)
                    nc.vector.reciprocal(rs[:], sums[:])
                    ye = WORK.tile([128, QCH], BF16, tag="ye", bufs=2)
                    nc.vector.scalar_tensor_tensor(
                        ye[:], yacc[:], 1.0, rs[:], op0=OP.mult, op1=OP.mult)
                    for u in range(2):
                        r0 = (2 * J + u) * 2 * HD + row_off
                        nc.gpsimd.dma_start(
                            a2ain[b][r0:r0 + 128, :],
                            ye[:, u * TSL:(u + 1) * TSL])

                def consume(J, i, cl, S):
                    P = WORK.tile([128, QCH], BF16, tag="P", bufs=4)
                    nc.scalar.activation(P[:, cl:QCH], S[:, cl:QCH], AF.Exp)
                    first, last = (i == 0), (i == (J + 1) * NKT - 1)
                    if first:
                        st[J] = (PS.tile([128, QCH], F32, tag="acc", bufs=2,
                                         name="sums"),
                                 PS.tile([128, QCH], F32, tag="yacc", bufs=2,
                                         name="yaccd"))
                    sums, yacc = st[J]
                    nc.tensor.matmul(sums[:, cl:QCH], ones[:], P[:, cl:QCH],
                                     start=first, stop=last,
                                     skip_group_check=True)
                    jj = (b * T) // KTILE + i
                    nc.tensor.matmul(
                        yacc[:, cl:QCH],
                        vtok[:, jj * KTILE:(jj + 1) * KTILE], P[:, cl:QCH],
                        start=first, stop=last, skip_group_check=True)
                    if last:
                        finalize(J, sums, yacc, 0)

                # ---- strided sparse head first (its A2A rows are ready
                # early; the dense J=3 eviction then gates the trigger) ----
                def emit_Ss(J):
                    S = PS.tile([NS, QCH], F32, tag="S", bufs=4)
                    nc.tensor.matmul(
                        S[:], ksT[:, b * NS:(b + 1) * NS],
                        qsT[:, b * T + J * QCH: b * T + (J + 1) * QCH],
                        start=True, stop=True)
                    return (J, S)

                def consume_s(J, S):
                    P = WORK.tile([NS, QCH], BF16, tag="Ps", bufs=2)
                    nc.scalar.activation(P[:], S[:], AF.Exp)
                    nc.vector.tensor_mul(
                        P[:], P[:], smask[:, J * QCH:(J + 1) * QCH])
                    sums = PS.tile([128, QCH], F32, tag="acc", bufs=2)
                    nc.tensor.matmul(sums[:], ones[0:NS, :], P[:],
                                     start=True, stop=True)
                    yacc = PS.tile([128, QCH], F32, tag="yacc", bufs=2)
                    nc.tensor.matmul(yacc[:], vs[:, b * HD:(b + 1) * HD],
                                     P[:], start=True, stop=True)
                    finalize(J, sums, yacc, 128)

                spend = []
                for J in range(NTCH):
                    spend.append(emit_Ss(J))
                    if len(spend) > 2:
                        consume_s(*spend.pop(0))
                while spend:
                    consume_s(*spend.pop(0))

                # ---- dense causal head ----
                sched = [(J, i) for J in range(NTCH)
                         for i in range((J + 1) * NKT)]
                pend = []
                for (J, i) in sched:
                    pend.append(emit_S(J, i))
                    if len(pend) > 3:
                        consume(*pend.pop(0))
                while pend:
                    consume(*pend.pop(0))

                nc.gpsimd.collective_compute(
                    "AllToAll", OP.bypass,
                    ins=[a2ain[b][:]], outs=[a2aout[b][:]],
                    replica_groups=[list(range(N_CORES))],
                )

            def proj_batch(b):
                ya = WORK.tile([128, CT, TSL], BF16, tag="ya", bufs=2)
                # SP queue: collective-gated load must not block the ACT
                # stream (the scheduler hoists it to the collective's
                # simulated completion, which is optimistic vs hardware)
                nc.sync.dma_start(
                    ya[:], a2aout[b][:].rearrange("(a p) n -> p a n", p=128))
                for oc in range(DIM // 512):
                    for tt in range(TSL // 128):
                        ps = PS.tile([128, 512], F32, tag="S", bufs=4)
                        for ci in range(CT):
                            nc.tensor.matmul(
                                ps[:], ya[:, ci, tt * 128:(tt + 1) * 128],
                                wp_sb[:, ci, oc * 512:(oc + 1) * 512],
                                start=(ci == 0), stop=(ci == CT - 1))
                        oe = WORK.tile([128, 512], F32, tag="oe", bufs=1)
                        nc.scalar.copy(oe[:], ps[:])
                        nc.scalar.dma_start(
                            out_tok[b * TSL + tt * 128: b * TSL + (tt + 1) * 128,
                                    oc * 512:(oc + 1) * 512], oe[:])

            # ---------------- schedule ----------------
            # SP queue priority order: x0 first (first-matmul critical path),
            # then the remaining b0 chunks and strided inputs
            qkv_chunk(0, 0)
            qkv_chunk(0, 1)
            nc.sync.dma_start(xs_sb[:], xsP[:])
            nc.sync.dma_start(wks_sb[:], wksP[:])
            nc.sync.dma_start(wvs_sb[:], wvsP[:])
            nc.sync.dma_start(ccS[:], ccS_d[:])
            nc.sync.dma_start(ssnS[:], ssnS_d[:])
            nc.sync.dma_start(smask[:], smask_d[:])
            qkv_chunk(0, 2)
            # strided kv early: its rope chain hides under chunk 3
            strided_phase()
            qkv_chunk(0, 3)
            nc.sync.dma_start(wp_sb[:], wpP[:])
            attn_batch(0)
            for tch in range(NTCH):
                qkv_chunk(1, tch)
            attn_batch(1)
            proj_batch(0)
            proj_batch(1)

    split_multi_waits(nc)
    return nc


_PROG_CACHE = {}


def _get_program():
    if "nc" not in _PROG_CACHE:
        _PROG_CACHE["nc"] = build_program()
    return _PROG_CACHE["nc"]


def _host_prep(x, w_attn, w_proj, q_gain, attn_temp):
    x = np.asarray(x, np.float32)
    w_attn = np.asarray(w_attn, np.float32)
    w_proj = np.asarray(w_proj, np.float32)
    q_gain = np.asarray(q_gain, np.float32)
    attn_temp = np.asarray(attn_temp, np.float32)

    BF = ml_dtypes.bfloat16
    xT = x.reshape(BT, DIM).T.astype(BF)                             # [DIM, BT]
    # [128, NCH, CT, XCH]: per-partition-contiguous chunk lines
    xP = np.ascontiguousarray(
        xT.reshape(CT, 128, NCH, XCH).transpose(1, 2, 0, 3))
    xs = x[:, ::STRIDE, :]                                           # [B, 46, DIM]
    xsP = _pmajor(xs.reshape(B * NS, DIM).T.astype(BF))

    g = (q_gain * attn_temp * SCALE).astype(np.float32)              # [H]
    wq = w_attn[:H * HD].reshape(H, HD, DIM)
    wq = wq * g[:, None, None]
    wk = w_attn[H * HD:(H + KV) * HD].reshape(KV, HD, DIM)
    wv = w_attn[(H + KV) * HD:].reshape(KV, HD, DIM)

    # w_proj^T with input dims permuted to A2A row order:
    # rank r contributes [dense head r | sparse head 8+r]
    perm = np.concatenate(
        [np.concatenate([np.arange(r * HD, (r + 1) * HD),
                         np.arange((8 + r) * HD, (9 + r) * HD)])
         for r in range(N_CORES)])
    wpP = _pmajor(w_proj.T[perm, :].astype(BF))

    in_maps = []
    for c in range(N_CORES):
        kva, kvb = c // 2, 4 + c // 2
        in_maps.append({
            "xP": xP,
            "xsP": xsP,
            "wqP": _pmajor(
                np.concatenate([wq[c], wq[8 + c]], axis=0).T.astype(BF)),
            "wkP": _pmajor(wk[kva].T.astype(BF)),
            "wvP": _pmajor(wv[kva].T.astype(BF)),
            "wksP": _pmajor(wk[kvb].T.astype(BF)),
            "wvsP": _pmajor(wv[kvb].T.astype(BF)),
            "wpP": wpP,
        })
    return in_maps


def run(x, w_attn, w_proj, q_gain, attn_temp, trace=False):
    nc = _get_program()
    in_maps = _host_prep(x, w_attn, w_proj, q_gain, attn_temp)
    res = run_bass_kernel_spmd(nc, in_maps, core_ids=list(range(N_CORES)),
                               trace=trace)
    out = np.empty((B, T, DIM), np.float32)
    for c in range(N_CORES):
        sh = res.results[c]["out_tok"]                               # [512, DIM]
        for b in range(B):
            out[b, c * TSL:(c + 1) * TSL, :] = sh[b * TSL:(b + 1) * TSL, :]
    return out, res


def kernel(x, w_attn, w_proj, q_gain, attn_temp):
    out, _ = run(x, w_attn, w_proj, q_gain, attn_temp, trace=False)
    return out
